# revision 1
# baseline (speedup 1.0000x reference)
"""Expert-parallel MoE layer for Trainium2 (8 NeuronCores, one expert per core).

Host side (numpy): router logits, exact top-2 dispatch, p0 weights, and the
scatter-add combine. Device side (Bass/Tile, SPMD over 8 cores): the dense FFN
y = gelu(x @ W1[e] + b1[e]) @ W2[e] over the tokens routed to expert e,
computed with fp16 operands (fp32 PSUM accumulation).

Per-core layout: F (the 4096-wide hidden dim) is processed in NQ=4 quarters
with W1/W2 quarter-slices resident in SBUF (double-buffered so the next
quarter's weights prefetch during compute); the whole fp16 xT stays resident;
y partials accumulate across quarters via DMA-accumulate into DRAM.
"""

import numpy as np

B, S, H, E, F = 4, 2048, 1024, 8, 4096
T = B * S
P = 128
NQ = 4              # F quarters (outer loop); W1q + W2q resident per quarter
FQ = F // NQ
TT = 512            # token group (GEMM1 moving free dim)
MIN_CAP = 2304      # >= max per-expert load for the fixed seed-0 input (~2182)

_cache = {}


def _spill_waits(nc, mybir, max_waits=1):
    """walrus CoreV2/V3 codegen rejects instructions with >1 semaphore wait
    ("Too many sync wait commands") — notably self-loading fp32/fp32r matmuls
    and DMACopy. Move excess waits onto same-engine no-ops inserted right
    before the instruction (sequencers run in order, so this is equivalent)."""
    for fn in nc.m.functions:
        for blk in fn.blocks:
            out = []
            changed = False
            for inst in blk.instructions:
                si = getattr(inst, "sync_info", None)
                if si is not None and len(si.on_wait) > max_waits:
                    spill = si.on_wait[: len(si.on_wait) - max_waits]
                    keep = si.on_wait[len(si.on_wait) - max_waits:]
                    for w in spill:
                        nop = mybir.InstNoOp(
                            name=nc.get_next_instruction_name(),
                            engine=inst.engine,
                            ins=[],
                            outs=[],
                        )
                        nop.sync_info = mybir.SyncInfo(on_wait=[w], on_update=[])
                        out.append(nop)
                    inst.sync_info = mybir.SyncInfo(on_wait=keep, on_update=si.on_update)
                    changed = True
                out.append(inst)
            if changed:
                blk.instructions = out


def _build(cap, w1_bufs=2):
    import concourse.bass as bass
    import concourse.mybir as mybir
    from concourse import tile

    F32 = mybir.dt.float32
    MDT = mybir.dt.float32r
    # all matmul operands fp16 (walrus forbids mixing fp32r with others):
    # halves DMA bytes + LDWEIGHTS time at ~2^-11 quantization cost
    SDT = mybir.dt.float16
    GELU = mybir.ActivationFunctionType.Gelu_apprx_tanh
    ADD = mybir.AluOpType.add

    nc = bass.Bass()
    xt = nc.declare_dram_parameter("xt", [H, cap], SDT, isOutput=False)
    w1 = nc.declare_dram_parameter("w1", [H, F], SDT, isOutput=False)
    w2 = nc.declare_dram_parameter("w2", [F, H], SDT, isOutput=False)
    b1s = nc.declare_dram_parameter("b1s", [P, F // P], F32, isOutput=False)
    y = nc.declare_dram_parameter("y", [cap, H], F32, isOutput=True)

    KH = H // P          # k-chunks over H (GEMM1 contraction)
    KFQ = FQ // P        # k-chunks over one F quarter (GEMM2 contraction)
    n_rows = cap // P
    groups = []
    o = 0
    while o < cap:
        tt = min(TT, cap - o)
        groups.append((o, tt))
        o += tt

    with tile.TileContext(nc) as tc:
        with (
            tc.tile_pool(name="w1p", bufs=w1_bufs) as w1p,
            tc.tile_pool(name="w2p", bufs=2) as w2p,
            tc.tile_pool(name="xp", bufs=1) as xp,
            tc.tile_pool(name="hp", bufs=1) as hp,
            tc.tile_pool(name="yp", bufs=1) as yp,
            tc.tile_pool(name="cst", bufs=1) as cst,
            tc.tile_pool(name="ps1", bufs=4, space="PSUM") as ps1,
            tc.tile_pool(name="ps2", bufs=4, space="PSUM") as ps2,
        ):
            def load_w1(q, split=False):
                # split=True (startup only): f-major halves on both HWDGE
                # fifos — the first half covers every k-chunk for fs=0..3, so
                # GEMM1's first four accumulation groups (32 matmuls, ~7us)
                # run while the second half is still in flight
                w1q = w1p.tile([P, KH, FQ], SDT, tag="w1q")
                src = w1[:, q * FQ:(q + 1) * FQ].rearrange("(c p) f -> p c f", p=P)
                if split:
                    nc.sync.dma_start(w1q[:, : KH // 2, :], src[:, : KH // 2, :])
                    nc.scalar.dma_start(w1q[:, KH // 2:, :], src[:, KH // 2:, :])
                else:
                    nc.sync.dma_start(w1q[:], src)
                return w1q

            def load_w2(q, split=False):
                # startup split is n-major: first half serves all n=0 output
                # tiles of GEMM2 for every k2
                w2q = w2p.tile([P, KFQ, H], SDT, tag="w2q")
                src = w2[q * FQ:(q + 1) * FQ, :].rearrange("(c p) h -> p c h", p=P)
                if split:
                    nc.scalar.dma_start(w2q[:, : KFQ // 2, :], src[:, : KFQ // 2, :])
                    nc.sync.dma_start(w2q[:, KFQ // 2:, :], src[:, KFQ // 2:, :])
                else:
                    nc.scalar.dma_start(w2q[:], src)
                return w2q

            # prologue: bias (tiny) + the first token group of x ahead of
            # the W1 halves; the rest of x (resident for the whole kernel in
            # fp16) follows once the startup-critical loads are queued
            b1t = cst.tile([P, F // P], F32)
            nc.scalar.dma_start(b1t[:], b1s[:])
            x_all = xp.tile([P, KH, cap], SDT)
            xsrc = xt.rearrange("(c p) t -> p c t", p=P)
            nc.scalar.dma_start(x_all[:, :, :TT], xsrc[:, :, :TT])
            w1q = load_w1(0, split=True)
            nc.scalar.dma_start(x_all[:, :, TT:], xsrc[:, :, TT:])
            w2q = None
            for q in range(NQ):
                for gi, (t0, tt) in enumerate(groups):
                    # GEMM1: hT[f, t] = sum_h W1[h, f] * xT[h, t], then gelu
                    hq = hp.tile([P, KFQ, TT], SDT, tag="hq")
                    for fs in range(KFQ):
                        pt = ps1.tile([P, TT], F32, tag="pt1")
                        for k in range(KH):
                            nc.tensor.matmul(
                                pt[:, :tt],
                                w1q[:, k, fs * P:(fs + 1) * P],
                                x_all[:, k, t0:t0 + tt],
                                start=(k == 0),
                                stop=(k == KH - 1),
                            )
                        c = q * KFQ + fs
                        nc.scalar.activation(
                            hq[:, fs, :tt], pt[:, :tt], GELU, bias=b1t[:, c:c + 1]
                        )
                    if q == 0 and gi == 0:
                        # W2 deliberately after GEMM1(group 0): its first use
                        # is GEMM2, so don't let it contend with W1/x at start
                        w2q = load_w2(0, split=True)
                    if gi == 0 and q + 1 < NQ:
                        w1_nxt = load_w1(q + 1)
                    if gi == 2 and q + 1 < NQ:
                        w2_nxt = load_w2(q + 1)
                    # GEMM2: y[t, h'] += sum_f hT[f, t] * W2[f, h']
                    rows = tt // P
                    stage = yp.tile([P, TT // P, H], F32, tag="stage")
                    for ms in range(rows):
                        for n in range(H // 512):
                            pt2 = ps2.tile([P, 512], F32, tag="pt2")
                            for k2 in range(KFQ):
                                nc.tensor.matmul(
                                    pt2[:],
                                    hq[:, k2, ms * P:(ms + 1) * P],
                                    w2q[:, k2, n * 512:(n + 1) * 512],
                                    start=(k2 == 0),
                                    stop=(k2 == KFQ - 1),
                                )
                            nc.vector.tensor_copy(
                                stage[:, ms, n * 512:(n + 1) * 512], pt2[:]
                            )
                    # y partial for this (quarter, group): write (q==0) or
                    # DMA-accumulate (q>0) into the y DRAM buffer
                    r0 = t0 // P
                    ydram = y.rearrange("(j p) h -> p j h", p=P)
                    if q == 0:
                        nc.sync.dma_start(
                            ydram[:, r0:r0 + rows, :], stage[:, :rows, :]
                        )
                    elif q == NQ - 1 and gi == len(groups) - 1:
                        # final writeback: per-row accum DMAs so most of it
                        # drains while the last matmul group still runs
                        for ms in range(rows):
                            nc.gpsimd.dma_start(
                                ydram[:, r0 + ms:r0 + ms + 1, :],
                                stage[:, ms:ms + 1, :],
                                accum_op=ADD,
                            )
                    else:
                        nc.gpsimd.dma_start(
                            ydram[:, r0:r0 + rows, :], stage[:, :rows, :], accum_op=ADD
                        )
                if q + 1 < NQ:
                    w1q, w2q = w1_nxt, w2_nxt

    import concourse.mybir as mybir_mod

    _spill_waits(nc, mybir_mod)
    return nc


def _route(x2d, Wr, br):
    """Top-2 routing, bit-matching the reference's decisions.

    Softmax is monotonic, so top-2-of-probs == top-2-of-logits, and the
    normalized top-1 weight p0 = p1/(p1+p2) == sigmoid(l1-l2) exactly (the
    softmax denominator cancels). Ordering ties are broken by lower index,
    same as jax.lax.top_k."""
    logits = x2d @ np.asarray(Wr, np.float32) + np.asarray(br, np.float32)
    order = np.argsort(-logits, axis=-1, kind="stable")
    i1 = order[:, 0].astype(np.int64)
    i2 = order[:, 1].astype(np.int64)
    r = np.arange(logits.shape[0])
    l1 = logits[r, i1].astype(np.float64)
    l2 = logits[r, i2].astype(np.float64)
    p0 = 1.0 / (1.0 + np.exp(l2 - l1))
    return i1, i2, p0.astype(np.float32)


def kernel(x, Wr, br, W1, b1, W2, b2):
    from concourse.bass_utils import run_bass_kernel_spmd

    x2d = np.ascontiguousarray(np.asarray(x, np.float32).reshape(T, H))
    W1 = np.asarray(W1, np.float32)
    b1 = np.asarray(b1, np.float32)
    W2 = np.asarray(W2, np.float32)
    b2 = np.asarray(b2, np.float32)

    i1, i2, p0 = _route(x2d, Wr, br)

    idxs = [np.flatnonzero((i1 == e) | (i2 == e)) for e in range(E)]
    max_cnt = max(len(ix) for ix in idxs)
    cap = max(MIN_CAP, -(-max_cnt // 256) * 256)

    key = cap
    if key not in _cache:
        _cache[key] = _build(cap)
    nc = _cache[key]

    xT = np.ascontiguousarray(x2d.T)  # [H, T]
    in_maps = []
    for e in range(E):
        ix = idxs[e]
        xte = np.zeros((H, cap), np.float32)
        xte[:, : len(ix)] = xT[:, ix]
        b1se = np.ascontiguousarray(b1[e].reshape(F // P, P).T)
        in_maps.append(
            {
                "xt": xte.astype(np.float16),
                "w1": np.ascontiguousarray(W1[e]).astype(np.float16),
                "w2": np.ascontiguousarray(W2[e]).astype(np.float16),
                "b1s": b1se,
            }
        )

    try:
        res = run_bass_kernel_spmd(nc, in_maps, list(range(E)))
    except Exception:
        import time as _time

        _time.sleep(10)
        res = run_bass_kernel_spmd(nc, in_maps, list(range(E)))

    out = np.zeros((T, H), np.float32)
    for e in range(E):
        ix = idxs[e]
        ye = res.results[e]["y"][: len(ix)]
        out[ix] += p0[ix, None] * (ye + b2[e][None, :])
    return out.reshape(B, S, H)

